# revision 9
# baseline (speedup 1.0000x reference)
"""Supervised-contrastive-style loss on 8 Trainium2 NeuronCores.

Math (reference):
    fn   = features / max(||features||, eps)           row-normalized
    sim  = (fn @ fn.T) / 0.5                           [N, N]
    pos  = labels[:, None] == labels[None, :]
    S_i  = sum_{j neg} exp(sim_ij) + (# pos in row i)  ("exp_neg")
    loss = mean over pos (i,j) of  softplus(log(S_i) - sim_ij)

Strategy:
  * Host sorts rows by label -> positive mask becomes block-diagonal.
    Each class c gets a fixed 1024-column "slot" (real cols + zero pads),
    so per-class column ranges are static and per-row positive work is a
    contiguous window.
  * Rows sharded across 8 cores by 128-row tiles (9 tiles/core).  One SPMD
    program; per-core differences (which row tiles, which class window) ride
    in as small int32 tensors consumed via register-indexed dynamic slices.
  * Per core: normalize + DMA-transpose to fnT [128d, 10240] bf16; for each
    of its row tiles matmul against all real columns, exp on PSUM in place
    with ACT accum_out giving per-class row sums (-> S_i exactly); then one
    softplus pass over the tile's own 1024-col class window with accum_out
    giving per-row loss sums (zero-pad columns corrected analytically).
  * Host sums per-row partials over real rows and divides by num_pos.
"""

import sys

if "/opt/trn_rl_repo" not in sys.path:
    sys.path.insert(0, "/opt/trn_rl_repo")

import numpy as np
import ml_dtypes

import concourse.bass as bass
import concourse.bacc as bacc
from concourse import mybir
from concourse.bass import ds
from concourse.bass_utils import run_bass_kernel_spmd
from concourse.tile import TileContext

P = 128
D = 128
N = 8192
NCLS = 10
SLOT = 1024                  # columns per class slot
NCOL = NCLS * SLOT           # padded column count (10240)
NTILE = NCOL // P            # 80 global 128-row tiles in padded layout
TPC = 9                      # row tiles per core
NCORES = 8
TEMP_SCALE = 2.0             # 1 / TEMPERATURE


def _build_program(widths):
    """Build the SPMD bass program. `widths` = per-class real column counts."""
    assert len(widths) == NCLS
    for w in widths:
        assert 512 < w <= SLOT, f"class width {w} outside (512, 1024]"

    nc = bacc.Bacc("TRN2", target_bir_lowering=False)
    bf16 = mybir.dt.bfloat16
    f32 = mybir.dt.float32

    frows = nc.declare_dram_parameter("frows", [P, NTILE, D], bf16, isOutput=False)
    meta = nc.declare_dram_parameter("meta", [TPC * 4], mybir.dt.int32, isOutput=False)
    pvec = nc.declare_dram_parameter("pvec", [P, TPC], f32, isOutput=False)
    padc = nc.declare_dram_parameter("padc", [P, TPC], f32, isOutput=False)
    out_loss = nc.declare_dram_parameter("loss9", [P, TPC], f32, isOutput=True)

    AF = mybir.ActivationFunctionType

    with TileContext(nc) as tc:
        with (
            tc.tile_pool(name="big", bufs=1) as big,
            tc.tile_pool(name="small", bufs=1) as small,
            tc.tile_pool(name="scratch", bufs=2) as scratch,
            tc.tile_pool(name="ps", bufs=3, space="PSUM") as ps,
        ):
            # ---------------- prep: load, normalize, transpose ----------------
            rows = big.tile([P, NTILE, D], bf16)        # raw rows, [p, t, d]
            nc.sync.dma_start(out=rows[:], in_=frows[:, :, :])

            meta_t = small.tile([1, TPC * 4], mybir.dt.int32)
            nc.sync.dma_start(out=meta_t[:], in_=meta[None, :])
            pvec_t = small.tile([P, TPC], f32)
            nc.sync.dma_start(out=pvec_t[:], in_=pvec[:, :])
            padc_t = small.tile([P, TPC], f32)
            nc.sync.dma_start(out=padc_t[:], in_=padc[:, :])

            sq = big.tile([P, NTILE, D], bf16)
            nc.vector.tensor_mul(sq[:], rows[:], rows[:])
            ss = small.tile([P, NTILE], f32)
            nc.vector.reduce_sum(ss[:], sq[:], axis=mybir.AxisListType.X)
            # ss += 1e-12: exact for real rows, avoids 1/0 on pad rows
            nc.vector.tensor_scalar_add(ss[:], ss[:], 1e-12)
            norm = small.tile([P, NTILE], f32)
            nc.scalar.activation(norm[:], ss[:], AF.Sqrt)
            rnorm = small.tile([P, NTILE], f32)
            nc.vector.reciprocal(rnorm[:], norm[:])

            fnrows = big.tile([P, NTILE, D], bf16)
            for t in range(NTILE):
                nc.vector.tensor_scalar_mul(
                    fnrows[:, t, :], rows[:, t, :], rnorm[:, t : t + 1]
                )

            fnT = big.tile([P, NCOL], bf16)             # [d, padded row index]
            for t in range(NTILE):
                nc.sync.dma_start_transpose(
                    fnT[:, t * P : (t + 1) * P], fnrows[:, t, :]
                )

            # stage this core's 9 stationary tiles at static addresses
            # (walrus can't do register offsets in ldweights)
            lhs_all = small.tile([P, TPC, P], bf16)
            for m in range(TPC):
                r = nc.vector.alloc_register(f"tcol{m}")
                nc.vector.reg_load(r, meta_t[0:1, 4 * m : 4 * m + 1])
                tcol = nc.vector.snap(r, donate=True, min_val=0, max_val=NCOL - P)
                nc.vector.tensor_copy(lhs_all[:, m, :], fnT[:, ds(tcol, P)])

            # ---------------- phase A: exp row sums per class ----------------
            # Sums[:, 10*m + c] = sum_{j in class c} exp(sim_ij), row tile m
            sums = small.tile([P, TPC * NCLS], f32)
            for m in range(TPC):
                lhsT = lhs_all[:, m, :]
                for c in range(NCLS):
                    w = widths[c]
                    pt = ps.tile([P, SLOT], f32, tag="mm")
                    nc.tensor.matmul(
                        pt[:, 0:512], lhsT, fnT[:, SLOT * c : SLOT * c + 512],
                        start=True, stop=True,
                    )
                    nc.tensor.matmul(
                        pt[:, 512:w], lhsT, fnT[:, SLOT * c + 512 : SLOT * c + w],
                        start=True, stop=True,
                    )
                    nc.scalar.activation(
                        pt[:, 0:w], pt[:, 0:w], AF.Exp, scale=TEMP_SCALE,
                        accum_out=sums[:, NCLS * m + c : NCLS * m + c + 1],
                    )

            # ---------------- combine: S = T - Tpos + P, L = ln(S) -----------
            t9 = small.tile([P, TPC], f32)
            nc.vector.reduce_sum(
                t9[:], sums[:].rearrange("p (m c) -> p m c", c=NCLS),
                axis=mybir.AxisListType.X,
            )
            tpos9 = small.tile([P, TPC], f32)
            for m in range(TPC):
                r = nc.vector.alloc_register(f"cls{m}")
                nc.vector.reg_load(r, meta_t[0:1, 4 * m + 3 : 4 * m + 4])
                ci = nc.vector.snap(r, donate=True, min_val=0, max_val=TPC * NCLS - 1)
                nc.vector.tensor_copy(tpos9[:, m : m + 1], sums[:, ds(ci, 1)])
            s9 = small.tile([P, TPC], f32)
            nc.vector.tensor_sub(s9[:], t9[:], tpos9[:])
            nc.vector.tensor_add(s9[:], s9[:], pvec_t[:])

            # e0 = device exp(0) (pad columns produce sim == 0 exactly);
            # lp1 = ln(e0 + S) is the per-row pad-column loss contribution
            e0 = small.tile([P, 1], f32)
            nc.vector.memset(e0[:], 0.0)
            nc.scalar.activation(e0[:], e0[:], AF.Exp, scale=TEMP_SCALE)
            q0 = small.tile([P, TPC], f32)
            nc.vector.tensor_scalar_add(q0[:], s9[:], e0[:, 0:1])
            lp1 = small.tile([P, TPC], f32)
            nc.scalar.activation(lp1[:], q0[:], AF.Ln)

            # ------- phase B: loss_ij = ln(E_ij + S_i) - sim_ij over window ---
            lnsum9 = small.tile([P, TPC], f32)
            simsum9 = small.tile([P, TPC], f32)
            for m in range(TPC):
                r0 = nc.tensor.alloc_register(f"w0_{m}")
                nc.tensor.reg_load(r0, meta_t[0:1, 4 * m + 1 : 4 * m + 2])
                w0 = nc.tensor.snap(r0, donate=True, min_val=0, max_val=NCOL - 512)
                r1 = nc.tensor.alloc_register(f"w1_{m}")
                nc.tensor.reg_load(r1, meta_t[0:1, 4 * m + 2 : 4 * m + 3])
                w1 = nc.tensor.snap(r1, donate=True, min_val=0, max_val=NCOL - 512)
                lhsT = lhs_all[:, m, :]
                pt = ps.tile([P, SLOT], f32, tag="mm")
                nc.tensor.matmul(pt[:, 0:512], lhsT, fnT[:, ds(w0, 512)],
                                 start=True, stop=True)
                nc.tensor.matmul(pt[:, 512:SLOT], lhsT, fnT[:, ds(w1, 512)],
                                 start=True, stop=True)
                ebuf = scratch.tile([P, SLOT], f32, tag="ebuf")
                nc.scalar.activation(ebuf[:], pt[:], AF.Exp, scale=TEMP_SCALE)
                # raw sim sum of the window (pad cols are exactly 0)
                nc.vector.reduce_sum(simsum9[:, m : m + 1], pt[:],
                                     axis=mybir.AxisListType.X)
                nc.vector.tensor_scalar_add(ebuf[:], ebuf[:], s9[:, m : m + 1])
                qlog = scratch.tile([P, SLOT], f32, tag="qlog")
                nc.scalar.activation(qlog[:], ebuf[:], AF.Ln,
                                     accum_out=lnsum9[:, m : m + 1])

            # loss rows = lnsum - 2*simsum_raw - padc * lp1
            loss9_t = small.tile([P, TPC], f32)
            nc.vector.tensor_scalar(loss9_t[:], simsum9[:], -TEMP_SCALE, None,
                                    op0=mybir.AluOpType.mult)
            nc.vector.tensor_add(loss9_t[:], loss9_t[:], lnsum9[:])
            corr = small.tile([P, TPC], f32)
            nc.vector.tensor_mul(corr[:], padc_t[:], lp1[:])
            nc.vector.tensor_sub(loss9_t[:], loss9_t[:], corr[:])

            nc.sync.dma_start(out=out_loss[:, :], in_=loss9_t[:])

    nc.finalize()
    return nc


_PROGRAM_CACHE = {}


def _get_program(widths):
    key = tuple(widths)
    if key not in _PROGRAM_CACHE:
        _PROGRAM_CACHE[key] = _build_program(key)
    return _PROGRAM_CACHE[key]


def _plan(labels):
    """Host-side layout plan from labels."""
    labels = np.asarray(labels).astype(np.int64)
    assert labels.shape == (N,)
    cnt = np.bincount(labels, minlength=NCLS)
    assert cnt.sum() == N and len(cnt) == NCLS

    perm = np.argsort(labels, kind="stable")
    num_pos = int((cnt.astype(np.int64) ** 2).sum())

    # real-containing global row tiles, in order
    tiles = []
    for c in range(NCLS):
        for k in range((int(cnt[c]) + P - 1) // P):
            tiles.append(8 * c + k)
    # pad to 72 with repeats (duplicates are ignored on output)
    while len(tiles) < TPC * NCORES:
        tiles.append(tiles[-1])
    assert len(tiles) == TPC * NCORES, "too many row tiles for 8x9 layout"
    return cnt, perm, num_pos, tiles


def _make_inputs(features, cnt, perm, tiles):
    fs = np.asarray(features, dtype=np.float32)[perm]
    fpad = np.zeros((NCOL, D), dtype=np.float32)
    off = 0
    for c in range(NCLS):
        n = int(cnt[c])
        fpad[SLOT * c : SLOT * c + n] = fs[off : off + n]
        off += n
    frows = (
        fpad.reshape(NTILE, P, D).transpose(1, 0, 2).astype(ml_dtypes.bfloat16).copy()
    )

    in_maps = []
    for i in range(NCORES):
        my = tiles[TPC * i : TPC * (i + 1)]
        meta = np.zeros(TPC * 4, dtype=np.int32)
        pv = np.zeros((P, TPC), dtype=np.float32)
        pc = np.zeros((P, TPC), dtype=np.float32)
        for m, g in enumerate(my):
            c = g // 8
            meta[4 * m + 0] = P * g
            meta[4 * m + 1] = SLOT * c
            meta[4 * m + 2] = SLOT * c + 512
            meta[4 * m + 3] = NCLS * m + c
            pv[:, m] = float(cnt[c])
            pc[:, m] = float(SLOT - int(cnt[c]))
        in_maps.append({"frows": frows, "meta": meta, "pvec": pv, "padc": pc})
    return in_maps


def _reduce_outputs(results, cnt, tiles, num_pos):
    seen = set()
    total = 0.0
    for i in range(NCORES):
        loss9 = np.asarray(results[i]["loss9"], dtype=np.float64)
        my = tiles[TPC * i : TPC * (i + 1)]
        for m, g in enumerate(my):
            if g in seen:
                continue
            seen.add(g)
            c = g // 8
            nreal = min(P, int(cnt[c]) - P * (g - 8 * c))
            if nreal <= 0:
                continue
            total += loss9[:nreal, m].sum()
    return np.float32(total / num_pos)


def run(features, labels, trace=False):
    cnt, perm, num_pos, tiles = _plan(labels)
    nc = _get_program(tuple(int(x) for x in cnt))
    in_maps = _make_inputs(features, cnt, perm, tiles)
    br = run_bass_kernel_spmd(
        nc, in_maps, core_ids=list(range(NCORES)), trace=trace
    )
    loss = _reduce_outputs(br.results, cnt, tiles, num_pos)
    return loss, br


def kernel(features, labels):
    loss, _ = run(features, labels, trace=False)
    return loss


def run_timed(features, labels, iters=32, warmup=4):
    """Estimate per-invocation device time by slope-timing repeated dispatches
    of the compiled SPMD executable (no NTFF profiling available under this
    axon client). Returns (loss, est_exec_ns)."""
    import time
    import jax
    from jax.sharding import Mesh, PartitionSpec, NamedSharding
    from jax.experimental.shard_map import shard_map
    from concourse import bass2jax

    cnt, perm, num_pos, tiles = _plan(labels)
    nc = _get_program(tuple(int(x) for x in cnt))
    in_maps = _make_inputs(features, cnt, perm, tiles)

    partition_name = nc.partition_id_tensor.name if nc.partition_id_tensor else None
    in_names, out_names, out_avals, zero_outs = [], [], [], []
    for alloc in nc.m.functions[0].allocations:
        if not isinstance(alloc, mybir.MemoryLocationSet):
            continue
        name = alloc.memorylocations[0].name
        if alloc.kind == "ExternalInput":
            if name != partition_name:
                in_names.append(name)
        elif alloc.kind == "ExternalOutput":
            out_names.append(name)
            shape = tuple(alloc.tensor_shape)
            dtype = mybir.dt.np(alloc.dtype)
            out_avals.append(jax.core.ShapedArray(shape, dtype))
            zero_outs.append(np.zeros(shape, dtype))
    n_params = len(in_names)
    n_outs = len(out_avals)
    in_names_all = in_names + out_names
    if partition_name is not None:
        in_names_all.append(partition_name)
    donate = tuple(range(n_params, n_params + n_outs))

    def _body(*args):
        operands = list(args)
        if partition_name is not None:
            operands.append(bass2jax.partition_id_tensor())
        outs = bass2jax._bass_exec_p.bind(
            *operands,
            out_avals=tuple(out_avals),
            in_names=tuple(in_names_all),
            out_names=tuple(out_names),
            lowering_input_output_aliases=(),
            sim_require_finite=True,
            sim_require_nnan=True,
            nc=nc,
        )
        return tuple(outs)

    devices = jax.devices()[:NCORES]
    mesh = Mesh(np.asarray(devices), ("core",))
    in_specs = (PartitionSpec("core"),) * (n_params + n_outs)
    out_specs = (PartitionSpec("core"),) * n_outs
    sharded = jax.jit(
        shard_map(_body, mesh=mesh, in_specs=in_specs, out_specs=out_specs,
                  check_rep=False),
        donate_argnums=donate, keep_unused=True,
    )
    per_core = [[np.asarray(m[name]) for name in in_names] for m in in_maps]
    sh = NamedSharding(mesh, PartitionSpec("core"))
    concat_in = [
        jax.device_put(
            np.concatenate([per_core[c][i] for c in range(NCORES)], axis=0), sh
        )
        for i in range(n_params)
    ]

    def zeros():
        return [np.zeros((NCORES * z.shape[0], *z.shape[1:]), z.dtype)
                for z in zero_outs]

    out = None
    for _ in range(warmup):
        out = sharded(*concat_in, *zeros())
        jax.block_until_ready(out)

    def timed(n):
        t0 = time.perf_counter()
        res = None
        for _ in range(n):
            res = sharded(*concat_in, *zeros())
        jax.block_until_ready(res)
        return time.perf_counter() - t0

    n1, n2 = max(2, iters // 4), iters
    t_small = min(timed(n1) for _ in range(3))
    t_big = min(timed(n2) for _ in range(3))
    est = (t_big - t_small) / (n2 - n1)

    out_np = np.asarray(out[out_names.index("loss9")]).reshape(
        NCORES, P, TPC
    )
    results = [{"loss9": out_np[c]} for c in range(NCORES)]
    loss = _reduce_outputs(results, cnt, tiles, num_pos)
    return loss, est * 1e9
